# revision 1
# baseline (speedup 1.0000x reference)
"""Trainium2 Bass kernel for nn_DynamicConv2d: per-sample dynamic conv.

  feat = x.mean(H,W); h1 = relu(feat@w1+b1); wgen = (h1@w2+b2) -> per-sample
  [COUT, CIN, 3, 3] conv weights; out[s] = conv2d(x[s], wgen[s], pad=1).

Sharding: batch B=32 across 8 cores (4 samples/core), MLP params replicated.

Per-core pipeline:
  - x arrives host-width-padded [4, 64, 128, 130] (zero side cols); loaded as
    two sample-pair images xp [(sp,ci)=128 partitions, 128, 130] fp32
  - feat: chunked DVE free-dim reduces overlapped with the x DMA
  - h1T = matmul(lhsT=w1/(H*W), rhs=feat4) -> Relu+b1 on ScalarE -> bf16
  - wgen: 72 chunk matmuls (bf16 in, fp32 psum; stationary h1T), two
    column-group tile_position packs per PSUM fill, rhs AP ordered
    [co16, ci_low32] so a DVE StreamTranspose lands ci on partitions;
    strided cross-quadrant copies assemble block-diagonal conv weights
    WT2 [(sp,ci), (sp,co), o] (off-diagonal zeros); + b2 via gathered tile
  - conv: per offset o one [K=128, M=128, N<=512] fp32r matmul per pair-tile
    (block-diag stationary covers both samples), 9 offsets accumulate in one
    PSUM bank; ScalarE drains; DMA out.
"""

import sys

for _p in ("/opt/trn_rl_repo",):
    if _p not in sys.path:
        sys.path.insert(0, _p)

from contextlib import ExitStack

import numpy as np

import concourse.bass as bass
import concourse.tile as tile
from concourse import bacc, mybir
from concourse.bass_utils import run_bass_kernel_spmd

F32 = mybir.dt.float32
F32R = mybir.dt.float32r
BF16 = mybir.dt.bfloat16

B, CIN, COUT, K, H, W = 32, 64, 64, 3, 128, 128
NCORES = 8
BSH = B // NCORES          # 4 samples per core
NPAIR = BSH // 2           # 2 sample-pairs per core
HID = 128                  # MLP hidden
JTOT = COUT * CIN * K * K  # 36864
NOFF = K * K               # 9
HW = H * W
WP = W + 2                 # width-padded image


def build_kernel_body(nc, tc, ctx, aps):
    x_ap = aps["x"]      # [BSH, CIN, H, WP]  (host width-padded)
    w1_ap = aps["w1"]    # [CIN, HID]
    b1_ap = aps["b1"]    # [HID, 1]
    w2_ap = aps["w2"]    # [HID, JTOT]
    b2_ap = aps["b2"]    # [JTOT]
    out_ap = aps["out"]  # [BSH, COUT, H, W]

    const = ctx.enter_context(tc.tile_pool(name="const", bufs=1))
    xpool = ctx.enter_context(tc.tile_pool(name="xpool", bufs=2))
    w2pool = ctx.enter_context(tc.tile_pool(name="w2pool", bufs=1))
    tpool = ctx.enter_context(tc.tile_pool(name="tpool", bufs=1))
    wtpool = ctx.enter_context(tc.tile_pool(name="wtpool", bufs=2))
    fpool = ctx.enter_context(tc.tile_pool(name="fpool", bufs=4))
    outp = ctx.enter_context(tc.tile_pool(name="outp", bufs=4))
    mlp_ps = ctx.enter_context(tc.tile_pool(name="mlp_ps", bufs=1, space="PSUM"))
    wg_ps = ctx.enter_context(tc.tile_pool(name="wg_ps", bufs=3, space="PSUM"))
    cv_ps = ctx.enter_context(tc.tile_pool(name="cv_ps", bufs=4, space="PSUM"))

    # ---- constants ----
    w1_sb = const.tile([CIN, HID], F32)
    nc.sync.dma_start(out=w1_sb, in_=w1_ap)
    w1s = const.tile([CIN, HID], F32)
    nc.scalar.mul(out=w1s, in_=w1_sb, mul=1.0 / HW)
    b1_sb = const.tile([HID, 1], F32)
    nc.sync.dma_start(out=b1_sb, in_=b1_ap)

    # b2 arrives host-prepped in block-diagonal conv layout
    # [(sp,ci), (sp,co), o] -- one clean contiguous DMA.
    b2T2 = const.tile([2 * CIN, 2 * COUT, NOFF], F32)
    nc.sync.dma_start(out=b2T2, in_=b2_ap)

    # ---- x loads + feat partial reduces (pair 0 first, then w2, then pair 1
    # -- HWDGE FIFO order makes conv-pair0's inputs land first) ----
    NXC = 4  # sub-DMAs per pair
    rows_per = H // NXC
    x2 = x_ap.rearrange("s c h w -> (s c) h w")
    xp_tiles = [None] * NPAIR
    fsum4 = const.tile([2 * CIN, BSH], F32)

    def load_pair(p):
        xp = xpool.tile([2 * CIN, H, WP], BF16, tag="xp", name=f"xp{p}")
        xp_tiles[p] = xp
        for c in range(NXC):
            r0 = c * rows_per
            nc.sync.dma_start(
                out=xp[:, r0 : r0 + rows_per, :],
                in_=x2[2 * p * CIN : (2 * p + 2) * CIN, r0 : r0 + rows_per, :],
            )

    def feat_pair(p):
        # per-chunk channel sums on ScalarE (accum_out); keeps DVE free for
        # the wgen StreamTranspose on the critical path
        xp = xp_tiles[p]
        fpart = fpool.tile([2 * CIN, NXC], F32, tag="fpart", name=f"fpart{p}")
        for c in range(NXC):
            r0 = c * rows_per
            if c % 2 == 0:
                ascr = fpool.tile(
                    [2 * CIN, rows_per * W], BF16, tag="ascr", name=f"ascr{p}_{c}"
                )
                nc.scalar.activation(
                    out=ascr,
                    in_=xp[:, r0 : r0 + rows_per, 1 : W + 1],
                    func=mybir.ActivationFunctionType.Copy,
                    accum_out=fpart[:, c : c + 1],
                )
            else:
                nc.vector.tensor_reduce(
                    out=fpart[:, c : c + 1],
                    in_=xp[:, r0 : r0 + rows_per, 1 : W + 1],
                    axis=mybir.AxisListType.XY,
                    op=mybir.AluOpType.add,
                )
        nc.vector.tensor_reduce(
            out=fsum4[:, 2 * p : 2 * p + 1],
            in_=fpart,
            axis=mybir.AxisListType.X,
            op=mybir.AluOpType.add,
        )

    NCB = 4
    CO_SL = COUT // NCB  # 16 co per slice
    SL = CO_SL * CIN * NOFF  # 9216
    COH = CO_SL // 2  # 8: co-half within a slice
    NW = COH * 32  # 256: chunk width (co-half x ci_low32)

    load_pair(0)
    load_pair(1)
    feat_pair(0)
    feat_pair(1)
    w2sl_tiles = []
    for cb in range(NCB):
        w2sl = w2pool.tile([HID, SL], BF16, tag=f"w2sl{cb}", name=f"w2sl{cb}")
        nc.sync.dma_start(out=w2sl, in_=w2_ap[:, cb * SL : (cb + 1) * SL])
        w2sl_tiles.append(w2sl)

    # ---- MLP (all 4 samples): h1T = relu(w1s.T @ feat4 + b1) -> bf16 ----
    feat4 = const.tile([CIN, BSH], F32)
    for p in range(NPAIR):
        nc.vector.tensor_copy(
            out=feat4[:, 2 * p : 2 * p + 1], in_=fsum4[0:CIN, 2 * p : 2 * p + 1]
        )
        nc.vector.tensor_copy(
            out=feat4[:, 2 * p + 1 : 2 * p + 2],
            in_=fsum4[CIN : 2 * CIN, 2 * p : 2 * p + 1],
        )
    h1_ps = mlp_ps.tile([HID, BSH], F32)
    nc.tensor.matmul(out=h1_ps, lhsT=w1s, rhs=feat4, start=True, stop=True)
    h1T32 = const.tile([HID, 32], BF16)
    nc.vector.memset(h1T32, 0.0)
    nc.scalar.activation(
        out=h1T32[:, 0:BSH],
        in_=h1_ps,
        func=mybir.ActivationFunctionType.Relu,
        bias=b1_sb,
        scale=1.0,
    )

    # ---- wgen: 4 column-groups per fill (2 co-blocks x 2 ci-halves) so the
    # StreamTranspose runs dense 128-partition ops ----
    wt_tiles = []
    for p in range(NPAIR):
        wt = wtpool.tile([2 * CIN, 2 * COUT, NOFF], BF16, tag="wt", name=f"wt{p}")
        wt_tiles.append(wt)
        nc.vector.memset(wt, 0.0)

    for cb in range(NCB):
        w2r = w2sl_tiles[cb].rearrange(
            "h (co ci o) -> h co ci o", co=CO_SL, ci=CIN, o=NOFF
        )
        tmid = tpool.tile(
            [2 * CIN, NOFF * NW], F32, tag="tmid", name=f"tmid{cb}"
        )
        for o in range(NOFF):
            wps = wg_ps.tile([2 * CIN, NW], F32, tag="wps", name=f"wps{cb}_{o}")
            for g in range(4):  # (co-half, ci-half)
                h, cih = g // 2, g % 2
                nc.tensor.matmul(
                    out=wps[32 * g : 32 * (g + 1), :],
                    lhsT=h1T32,
                    rhs=w2r[
                        :, COH * h : COH * (h + 1), 32 * cih : 32 * (cih + 1), o
                    ],
                    start=True,
                    stop=True,
                    tile_position=(0, 32 * g),
                )
            # T[32g + cil, 32co + s] = wps[32g + s, 32co + cil]
            nc.vector.transpose(out=tmid[:, o * NW : (o + 1) * NW], in_=wps)
        tr = tmid.rearrange("p (o co s) -> p o co s", o=NOFF, co=COH, s=32)
        for s in range(BSH):
            pr, sp = s // 2, s % 2
            for h in range(2):
                dst = wt_tiles[pr][
                    sp * CIN : (sp + 1) * CIN,
                    sp * COUT + cb * CO_SL + h * COH :
                    sp * COUT + cb * CO_SL + (h + 1) * COH,
                    :,
                ].rearrange("p co o -> p o co")
                nc.vector.tensor_copy(
                    out=dst, in_=tr[64 * h : 64 * (h + 1), :, :, s]
                )

    for p in range(NPAIR):
        nc.vector.tensor_add(wt_tiles[p], wt_tiles[p], b2T2)

    # ---- conv ----
    TROWS = 4
    NT = H // TROWS
    out2 = out_ap.rearrange("s c h w -> (s c) (h w)")
    # center offset first so start=True covers every psum element
    off_order = [4, 0, 1, 2, 3, 5, 6, 7, 8]
    for p in range(NPAIR):
        wt = wt_tiles[p]
        xr = xp_tiles[p]  # [q, H, WP]
        TB = 2  # conv tiles batched per output DMA
        for tb in range(NT // TB):
            ost = outp.tile(
                [2 * CIN, TB * TROWS * W], F32, tag="ost", name=f"ost{p}_{tb}"
            )
            for tt in range(TB):
                t = tb * TB + tt
                h0 = t * TROWS
                cvp = cv_ps.tile(
                    [2 * CIN, TROWS * W], F32, tag="cvp", name=f"cvp{p}_{t}"
                )
                for i, o in enumerate(off_order):
                    dy, dx = o // 3, o % 3
                    h_lo = max(h0, 1 - dy)
                    h_hi = min(h0 + TROWS, H + 1 - dy)
                    nr = h_hi - h_lo
                    xr0 = h_lo + dy - 1
                    nc.tensor.matmul(
                        out=cvp[:, (h_lo - h0) * W : (h_hi - h0) * W],
                        lhsT=wt[:, :, o],
                        rhs=xr[:, xr0 : xr0 + nr, dx : dx + W],
                        start=(i == 0),
                        stop=(i == len(off_order) - 1),
                    )
                nc.scalar.copy(
                    out=ost[:, tt * TROWS * W : (tt + 1) * TROWS * W], in_=cvp
                )
            nc.sync.dma_start(
                out=out2[
                    2 * p * CIN : (2 * p + 2) * CIN,
                    tb * TB * TROWS * W : (tb + 1) * TB * TROWS * W,
                ],
                in_=ost,
            )


_CACHE = {}


def build_nc():
    if "nc" in _CACHE:
        return _CACHE["nc"], _CACHE["aps"]
    nc = bacc.Bacc("TRN2", debug=False, num_devices=NCORES)
    aps = {
        "x": nc.dram_tensor("x", [BSH, CIN, H, WP], BF16, kind="ExternalInput").ap(),
        "w1": nc.dram_tensor("w1", [CIN, HID], F32, kind="ExternalInput").ap(),
        "b1": nc.dram_tensor("b1", [HID, 1], F32, kind="ExternalInput").ap(),
        "w2": nc.dram_tensor("w2", [HID, JTOT], BF16, kind="ExternalInput").ap(),
        "b2": nc.dram_tensor(
            "b2", [2 * CIN, 2 * COUT, NOFF], F32, kind="ExternalInput"
        ).ap(),
        "out": nc.dram_tensor("out", [BSH, COUT, H, W], F32, kind="ExternalOutput").ap(),
    }
    with tile.TileContext(nc) as tc, ExitStack() as ctx:
        build_kernel_body(nc, tc, ctx, aps)
    nc.compile()
    _CACHE["nc"] = nc
    _CACHE["aps"] = aps
    return nc, aps


def make_in_maps(x, w1, b1, w2, b2):
    import ml_dtypes
    x = np.asarray(x, dtype=np.float32)
    xpad = np.zeros((B, CIN, H, WP), dtype=ml_dtypes.bfloat16)
    xpad[:, :, :, 1 : W + 1] = x.astype(ml_dtypes.bfloat16)
    w1 = np.ascontiguousarray(np.asarray(w1, dtype=np.float32))
    b1 = np.ascontiguousarray(np.asarray(b1, dtype=np.float32)).reshape(HID, 1)
    w2 = np.ascontiguousarray(
        np.asarray(w2, dtype=np.float32).astype(ml_dtypes.bfloat16)
    )
    b2v = np.asarray(b2, dtype=np.float32).reshape(COUT, CIN, NOFF)
    b2t = np.zeros((2 * CIN, 2 * COUT, NOFF), dtype=np.float32)
    for sp in range(2):
        b2t[sp * CIN : (sp + 1) * CIN, sp * COUT : (sp + 1) * COUT, :] = (
            b2v.transpose(1, 0, 2)
        )
    b2 = np.ascontiguousarray(b2t)
    in_maps = []
    for c in range(NCORES):
        in_maps.append(
            {
                "x": np.ascontiguousarray(xpad[c * BSH : (c + 1) * BSH]),
                "w1": w1,
                "b1": b1,
                "w2": w2,
                "b2": b2,
            }
        )
    return in_maps


def kernel(x, w1, b1, w2, b2, _trace=False, _results_out=None):
    nc, _ = build_nc()
    in_maps = make_in_maps(x, w1, b1, w2, b2)
    res = run_bass_kernel_spmd(
        nc, in_maps, core_ids=list(range(NCORES)), trace=_trace
    )
    if _results_out is not None:
        _results_out.append(res)
    out = np.concatenate([r["out"] for r in res.results], axis=0)
    return out


if __name__ == "__main__":
    rng = np.random.default_rng(0)
    ins = {
        "x": rng.standard_normal((B, CIN, H, W)).astype(np.float32),
        "w1": (rng.standard_normal((CIN, HID)) * 0.05).astype(np.float32),
        "b1": (rng.standard_normal((HID,)) * 0.05).astype(np.float32),
        "w2": (rng.standard_normal((HID, JTOT)) * 0.05).astype(np.float32),
        "b2": (rng.standard_normal((JTOT,)) * 0.05).astype(np.float32),
    }
    out = kernel(**ins)
    print("out", out.shape, out.dtype, np.abs(out).mean())

